# revision 35
# baseline (speedup 1.0000x reference)
"""Trainium2 Bass kernel for a single transformer encoder layer.

Problem: B=4, S=2048, D=512, H=8 (dk=64), DFF=2048, f32 I/O.
Sharding: 8 cores = (batch b, token-half). Each core computes the full
layer for its 1024 tokens; K/V are computed for the whole 2048-token
batch on both cores of a pair (duplicated, zero communication).

v3: fp8(e4m3) DoubleRow matmuls + software-pipelined schedule.
  - The softmax EXP on the ACT engine (1024 elems/partition @ 1.2GHz
    ~= 1.1us x 128 instrs) is the pacing resource; everything else is
    emitted as fine-grained (<~0.5us PE) "fill" tasks drained one per
    t2 iteration inside the attention loops so the PE never stalls the
    exp pipeline and never idles long enough for HAM to re-throttle.
  - block-0 attention shadow: V projection, K/Q(hs=1) projections,
    Q(block1) projections, W1/W2 DMAs, warm-keeper matmuls.
  - block-1 attention shadow: block-0 Z-normalize, post-attn (Wo,
    residual+LN1, transposes), FFN1, FFN2+LN2+store.
  - tail: block-1 post/FFN only.
  - fp8 scales (power-of-2): xT8 = 2x, w{q,k,v,1} /2, wo*4,
    ctxT8 = 64*ctx (rz = 64/Z via ACT bias ln64), w2 as-is, attn-out
    evict *2^-8 fused with the residual (xo has bo pre-added),
    1/sqrt(dk) folded into the EXP scale immediate.
  - scores: K^T/Q^T fp8 strip-mapped ([32, 2(plane), cols] per head,
    head h on partition strip 32*(h%4)); the two DoubleRow score
    matmuls of a head-pair run concurrently on disjoint row strips.
"""

from collections import deque
from contextlib import ExitStack

import numpy as np
import ml_dtypes

import concourse.bass as bass
import concourse.tile as tile
from concourse import mybir, bacc
from concourse.bass_utils import run_bass_kernel_spmd
from concourse.masks import make_identity

F32 = mybir.dt.float32
BF16 = mybir.dt.bfloat16
FP8 = mybir.dt.float8e4
AF = mybir.ActivationFunctionType
OP = mybir.AluOpType
DRM = mybir.MatmulPerfMode.DoubleRow

B, S, D = 4, 2048, 512
H, DK, DFF = 8, 64, 2048
EPS = 1e-5
P = 128
T1 = 1024          # own tokens per core
NCORES = 8

KD = D // P        # 4   d-tiles
NT2 = S // P       # 16  t2 tiles (context tokens)
NT1 = T1 // P      # 8   t1 tiles (own tokens)
NPAIR = H // 2     # 4   head pairs
NDFF = DFF // P    # 16  dff tiles
NB1 = T1 // 512    # 2   own-token 512-blocks
NBS = S // 512     # 4   context 512-blocks
DV1 = DK + 1       # 65  V columns incl the ones column
LN64 = float(np.log(64.0))


def emit(ctx: ExitStack, tc, io):
    nc = tc.nc

    xT, xTo, xo = io["xT"], io["xTo"], io["xo"]
    wq, wk, wv, wo, w1, w2 = io["wq"], io["wk"], io["wv"], io["wo"], io["w1"], io["w2"]
    out = io["out"]

    const = ctx.enter_context(tc.tile_pool(name="const", bufs=1))
    persist = ctx.enter_context(tc.tile_pool(name="persist", bufs=1))
    exp_pool = ctx.enter_context(tc.tile_pool(name="exp", bufs=4))
    cxu_pool = ctx.enter_context(tc.tile_pool(name="cxu", bufs=10))
    work = ctx.enter_context(tc.tile_pool(name="work", bufs=2))
    stat = ctx.enter_context(tc.tile_pool(name="stat", bufs=4))
    norm = ctx.enter_context(tc.tile_pool(name="norm", bufs=1))
    xo_pool = ctx.enter_context(tc.tile_pool(name="xo", bufs=2))
    out_pool = ctx.enter_context(tc.tile_pool(name="out", bufs=2))

    mm_ps = ctx.enter_context(tc.tile_pool(name="mm_ps", bufs=1, space="PSUM"))
    ff_ps = ctx.enter_context(tc.tile_pool(name="ff_ps", bufs=1, space="PSUM"))
    sc_ps = ctx.enter_context(tc.tile_pool(name="sc_ps", bufs=2, space="PSUM"))
    ctx_ps = ctx.enter_context(tc.tile_pool(name="ctx_ps", bufs=1, space="PSUM"))

    # ---- persistent SBUF arrays ----
    wk_sb = persist.tile([P, KD, D], FP8, tag="wk")
    nc.scalar.dma_start(wk_sb[:], wk[:, :].rearrange("(k p) m -> p k m", p=P))
    xT_sb = persist.tile([P, KD, S], FP8, tag="xT")
    for k in range(KD):
        nc.sync.dma_start(
            xT_sb[:, k, :], xT[:, :].rearrange("(k p) t -> p k t", p=P)[:, k, :]
        )
    wq_sb = persist.tile([P, KD, D], FP8, tag="wq")
    nc.scalar.dma_start(wq_sb[:], wq[:, :].rearrange("(k p) m -> p k m", p=P))
    wv_sb = persist.tile([P, KD, D], FP8, tag="wv")
    nc.scalar.dma_start(wv_sb[:], wv[:, :].rearrange("(k p) m -> p k m", p=P))
    xTo_sb = persist.tile([P, KD, T1], FP8, tag="xTo")
    for k in range(KD):
        nc.sync.dma_start(
            xTo_sb[:, k, :], xTo[:, :].rearrange("(k p) t -> p k t", p=P)[:, k, :]
        )
    wo_sb = persist.tile([P, KD, D], FP8, tag="wo")
    nc.scalar.dma_start(wo_sb[:], wo[:, :].rearrange("(k p) m -> p k m", p=P))

    # K^T/Q^T bf16, head-pairs stacked on the 128 partitions
    kt_sb = persist.tile([P, NPAIR, S], BF16, tag="kt")
    qt_sb = persist.tile([P, NPAIR, T1], BF16, tag="qt")
    # V with per-head ones column: [t2 128, t2tile, head, 65]
    ve_sb = persist.tile([P, NT2, H, DV1], BF16, tag="ve")
    nc.vector.memset(ve_sb[:, :, :, DK:DV1], 1.0)
    ctxT_sb = persist.tile([P, NPAIR, T1], FP8, tag="ctxT")
    x1_sb = persist.tile([P, NT1, D], BF16, tag="x1")
    x1T_sb = persist.tile([P, KD, T1], FP8, tag="x1T")
    h1T_sb = persist.tile([P, NDFF, T1], BF16, tag="h1T")

    # ---- constants ----
    ident_sb = const.tile([P, P], BF16)
    make_identity(nc, ident_sb[:])
    eps_sb = const.tile([P, 1], F32)
    nc.vector.memset(eps_sb[:], EPS)
    ln64_sb = const.tile([8, 1], F32)
    nc.vector.memset(ln64_sb[:], LN64)
    # indicator for the 1/Z partition-broadcast: row h covers columns of head h
    ind8_sb = const.tile([8, D], BF16)
    nc.gpsimd.dma_start(ind8_sb[:], io["ind8"][:, :])

    # per-partition bias tiles (feature-major evictions; bq/bk permuted)
    bqt = const.tile([P, KD], F32)
    nc.gpsimd.dma_start(bqt[:], io["bq"][:].rearrange("(m p) -> p m", p=P))
    bkt = const.tile([P, KD], F32)
    nc.gpsimd.dma_start(bkt[:], io["bk"][:].rearrange("(m p) -> p m", p=P))
    b1t = const.tile([P, NDFF], F32)
    nc.gpsimd.dma_start(b1t[:], io["b1"][:].rearrange("(m p) -> p m", p=P))

    # free-axis broadcast tiles (token-major ops)
    def bc_tile(name):
        t = const.tile([P, D], BF16, tag=f"bc_{name}")
        a = io[name][:]
        bcast = bass.AP(tensor=a.tensor, offset=a.offset, ap=[[0, P]] + list(a.ap))
        nc.gpsimd.dma_start(t[:], bcast)
        return t

    bvb = bc_tile("bv")   # V-proj evictions need this early
    b2b = bc_tile("b2")
    g1b = bc_tile("g1")
    be1b = bc_tile("be1")
    g2b = bc_tile("g2")
    be2b = bc_tile("be2")

    # scratch psum: alternate between the two 1-buf pools so back-to-back
    # tasks (head/tail, no slot pacing) pipeline MM vs eviction
    _sc_i = [0]

    def scratch_ps(shape, dtype):
        if ff_live:
            return mm_ps.tile(shape, dtype, tag="mm", name="scr")
        _sc_i[0] ^= 1
        return (mm_ps, ff_ps)[_sc_i[0]].tile(
            shape, dtype, tag=("mm", "ff")[_sc_i[0]], name="scr"
        )

    # ---- projection task units (fp8 DoubleRow over k-tile pairs) ----
    def proj_k(t, nb, on_act=False):
        ps = scratch_ps([P, 512], F32)
        for j in range(2):
            nc.tensor.matmul(
                ps[:],
                wk_sb[:, 2 * j:2 * j + 2, t * P:(t + 1) * P],
                xT_sb[:, 2 * j:2 * j + 2, nb * 512:(nb + 1) * 512],
                start=(j == 0), stop=(j == 1), perf_mode=DRM,
            )
        dst = kt_sb[:, t, nb * 512:(nb + 1) * 512]
        if on_act:
            nc.scalar.activation(dst, ps[:], AF.Identity, bias=bkt[:, t:t + 1])
        else:
            nc.vector.tensor_scalar(dst, ps[:], bkt[:, t:t + 1], None, op0=OP.add)

    def proj_q(t, nb, on_act=False):
        ps = scratch_ps([P, 512], F32)
        for j in range(2):
            nc.tensor.matmul(
                ps[:],
                wq_sb[:, 2 * j:2 * j + 2, t * P:(t + 1) * P],
                xTo_sb[:, 2 * j:2 * j + 2, nb * 512:(nb + 1) * 512],
                start=(j == 0), stop=(j == 1), perf_mode=DRM,
            )
        dst = qt_sb[:, t, nb * 512:(nb + 1) * 512]
        if on_act:
            nc.scalar.activation(dst, ps[:], AF.Identity, bias=bqt[:, t:t + 1])
        else:
            nc.vector.tensor_scalar(dst, ps[:], bqt[:, t:t + 1], None, op0=OP.add)

    def proj_v(i):
        ps = scratch_ps([P, 512], F32)
        for j in range(2):
            nc.tensor.matmul(
                ps[:],
                xT_sb[:, 2 * j:2 * j + 2, i * P:(i + 1) * P],
                wv_sb[:, 2 * j:2 * j + 2, :],
                start=(j == 0), stop=(j == 1), perf_mode=DRM,
            )
        nc.vector.tensor_tensor(
            ve_sb[:, i, :, 0:DK],
            ps[:].rearrange("p (h d) -> p h d", h=H),
            bvb[:].rearrange("p (h d) -> p h d", h=H),
            OP.add,
        )

    def layer_norm(r, gb, beb, dest, eng=None):
        """dest = LN(r)*g + be; r is f32 SBUF [128, D]."""
        eng = eng or nc.gpsimd
        st = stat.tile([P, 6], F32, tag="st")
        nc.vector.bn_stats(st[:], r[:])
        mv = stat.tile([P, 2], F32, tag="mv")
        nc.vector.bn_aggr(mv[:], st[:])
        lnv = stat.tile([P, 1], F32, tag="lnv")
        nc.scalar.activation(lnv[:], mv[:, 1:2], AF.Ln, bias=eps_sb[:, 0:1])
        rstd = stat.tile([P, 1], F32, tag="rstd")
        nc.scalar.activation(rstd[:], lnv[:], AF.Exp, scale=-0.5)
        xc = work.tile([P, D], F32, tag="xc")
        nc.vector.tensor_scalar(
            xc[:], r[:], mv[:, 0:1], rstd[:], op0=OP.subtract, op1=OP.mult
        )
        xg = work.tile([P, D], F32, tag="xg")
        eng.tensor_tensor(xg[:], xc[:], gb[:], OP.mult)
        eng.tensor_tensor(dest, xg[:], beb[:], OP.add)

    post_stats = {}

    def post_attn1(t1t):
        ao = scratch_ps([P, 512], F32)
        xo_t = xo_pool.tile([P, D], F32)
        nc.sync.dma_start(xo_t[:], xo[t1t * P:(t1t + 1) * P, :])
        for k in range(2):
            nc.tensor.matmul(
                ao[:], ctxT_sb[:, 2 * k:2 * k + 2, t1t * P:(t1t + 1) * P],
                wo_sb[:, 2 * k:2 * k + 2, :],
                start=(k == 0), stop=(k == 1), perf_mode=DRM,
            )
        rslot = x1_sb[:, t1t, :]
        nc.vector.scalar_tensor_tensor(
            rslot, ao[:], 2.0 ** -8, xo_t[:], OP.mult, OP.add
        )
        st = stat.tile([P, 6], F32, tag="st")
        nc.vector.bn_stats(st[:], rslot)
        mv = stat.tile([P, 2], F32, tag="mv")
        nc.vector.bn_aggr(mv[:], st[:])
        post_stats[t1t] = mv

    def post_attn2a(t1t, eng=None):
        """LN1 normalize+affine (in the x1 slot)."""
        eng = eng or nc.gpsimd
        mv = post_stats.pop(t1t)
        rslot = x1_sb[:, t1t, :]
        lnv = stat.tile([P, 1], F32, tag="lnv")
        nc.scalar.activation(lnv[:], mv[:, 1:2], AF.Ln, bias=eps_sb[:, 0:1])
        rstd = stat.tile([P, 1], F32, tag="rstd")
        nc.scalar.activation(rstd[:], lnv[:], AF.Exp, scale=-0.5)
        xc = work.tile([P, D], F32, tag="xc")
        nc.vector.tensor_scalar(
            xc[:], rslot, mv[:, 0:1], rstd[:], op0=OP.subtract, op1=OP.mult
        )
        xg = work.tile([P, D], F32, tag="xg")
        eng.tensor_tensor(xg[:], xc[:], g1b[:], OP.mult)
        eng.tensor_tensor(rslot, xg[:], be1b[:], OP.add)

    def post_attn2b(t1t, j, evict_act=False):
        """transpose(2*x1) -> x1T fp8, one 128-col tile."""
        tp = scratch_ps([P, P], BF16)
        nc.tensor.transpose(
            tp[:], x1_sb[:, t1t, j * P:(j + 1) * P], ident_sb[:]
        )
        if evict_act:
            nc.scalar.mul(x1T_sb[:, j, t1t * P:(t1t + 1) * P], tp[:], 2.0)
        else:
            nc.vector.tensor_scalar(
                x1T_sb[:, j, t1t * P:(t1t + 1) * P], tp[:], 2.0, None,
                op0=OP.mult,
            )

    def ffn1(t1b, m, on_act=False, off=0, width=512):
        lo = t1b * 512 + off
        ps = scratch_ps([P, 512], F32)
        for j in range(2):
            nc.tensor.matmul(
                ps[:, 0:width],
                w1_holder[0][:, 2 * j:2 * j + 2, m * P:(m + 1) * P],
                x1T_sb[:, 2 * j:2 * j + 2, lo:lo + width],
                start=(j == 0), stop=(j == 1), perf_mode=DRM,
            )
        # h1 = relu(ps + b1); ACT in the tail block (ACT idle there)
        if on_act:
            nc.scalar.activation(
                h1T_sb[:, m, lo:lo + width], ps[:, 0:width], AF.Relu,
                bias=b1t[:, m:m + 1],
            )
        else:
            nc.vector.tensor_scalar(
                h1T_sb[:, m, lo:lo + width], ps[:, 0:width],
                b1t[:, m:m + 1], 0.0, op0=OP.add, op1=OP.max,
            )

    ff_live = {}

    def ffn2_mm(t1t, k0, nk, acc="ff"):
        if k0 == 0:
            pool = ff_ps if acc == "ff" else mm_ps
            ff_live[t1t] = pool.tile([P, 512], F32, tag=acc, name="ffacc")
        ff = ff_live[t1t]
        for k in range(k0, k0 + nk):
            nc.tensor.matmul(
                ff[:],
                h1T_sb[:, k, t1t * P:(t1t + 1) * P],
                w2_holder[0][:, k, :],
                start=(k == 0), stop=(k == NDFF - 1),
            )

    def ffn2_fin(t1t, eng=None):
        ff = ff_live.pop(t1t)
        r = work.tile([P, D], F32, tag="r2")
        nc.vector.scalar_tensor_tensor(
            r[:], ff[:], 1.0, x1_sb[:, t1t, :], OP.mult, OP.add
        )
        nc.vector.tensor_tensor(r[:], r[:], b2b[:], OP.add)
        o = out_pool.tile([P, D], F32)
        layer_norm(r, g2b, be2b, o[:], eng=eng or nc.vector)
        nc.sync.dma_start(out[t1t * P:(t1t + 1) * P, :], o[:])

    w1_holder = [None]
    w2_holder = [None]

    def load_w1():
        w1_holder[0] = persist.tile([P, KD, DFF], FP8, tag="xT", name="w1_sb")
        nc.scalar.dma_start(
            w1_holder[0][:], w1[:, :].rearrange("(k p) m -> p k m", p=P)
        )

    def load_w2():
        w2_holder[0] = persist.tile([P, NDFF, D], BF16, tag="xTo", name="w2_sb")
        nc.scalar.dma_start(
            w2_holder[0][:], w2[:, :].rearrange("(k p) m -> p k m", p=P)
        )

    # ---- fill-task queue ----
    fills = deque()

    def drain(n=1):
        for _ in range(min(n, len(fills))):
            fills.popleft()()

    # ---- head: minimum work to start attention pair 0 of block 0 ----
    # pre-warm the PE (HAM un-throttles after ~3.4us of sustained work) on
    # a zeroed scrap tile while the input DMAs land
    scrap = const.tile([P, 512], BF16, name="scrap")
    nc.vector.memset(scrap[:], 0.0)
    for _ in range(8):
        ps = scratch_ps([P, 512], F32)
        nc.tensor.matmul(ps[:], scrap[:, 0:128], scrap[:], start=True, stop=True)
    # pair 0 needs only kt tile 0, qt tile (0, block0), and ve[0..] leads
    for nb in range(NBS):
        proj_k(0, nb, on_act=True)
    proj_v(0)
    proj_v(1)
    proj_q(0, 0, on_act=True)
    for i in range(2, 6):
        proj_v(i)

    def z_normalize(h, t1s, rzall, cxu, in_tail=False):
        pair, odd = divmod(h, 2)
        if in_tail:
            bch = sc_ps.tile([64, 512], F32, tag="s", name="bch")
        else:
            bch = mm_ps.tile([64, 512], F32, tag="mm", name="bch")
        nc.tensor.matmul(
            bch[:], ind8_sb[:, h * DK:(h + 1) * DK], rzall[:, :],
            start=True, stop=True,
        )
        if not odd:
            nc.vector.tensor_tensor(
                ctxT_sb[0:64, pair, t1s], cxu[h][:], bch[:], OP.mult
            )
        else:
            stg = work.tile([64, 512], FP8, tag="stg")
            nc.vector.tensor_tensor(stg[:], cxu[h][:], bch[:], OP.mult)
            nc.gpsimd.dma_start(ctxT_sb[64:128, pair, t1s], stg[:])

    # ---- attention (t1-block outer so downstream work pipelines) ----
    for t1b in range(NB1):
        t1s = slice(t1b * 512, (t1b + 1) * 512)
        zall = norm.tile([8, 512], F32, tag=f"zall{t1b % 2}")
        cxu = {}

        if t1b == 0:
            # block-0 shadow, drained during pair 0: late V tiles (each >=5
            # slots ahead of its ctx) then pair-1's K/Q projections
            for i in range(6, NT2):
                fills.append(lambda i=i: proj_v(i))
            for nb in range(NBS):
                fills.append(lambda nb=nb: proj_k(1, nb))
            fills.append(lambda: proj_q(1, 0))
        else:
            # block-1 shadow: block-0 post-attn + FFN
            for t in range(4):
                fills.append(lambda t=t: post_attn1(t))
            for t in range(4):
                fills.append(lambda t=t: post_attn2a(t))
                for j in range(KD):
                    fills.append(lambda t=t, j=j: post_attn2b(t, j))
            for m in range(NDFF):
                fills.append(lambda m=m: ffn1(0, m))
            for t in range(4):
                for k0 in range(0, NDFF, 2):
                    fills.append(lambda t=t, k0=k0: ffn2_mm(t, k0, 2))
                if t < 2:
                    fills.append(lambda t=t: ffn2_fin(t))

        slot_i = [0]

        def drain_slot():
            slot_i[0] += 1
            if t1b == 1 and slot_i[0] <= 3:
                return
            if fills:
                drain(1)

        for pair in range(NPAIR):
            hA, hB = 2 * pair, 2 * pair + 1
            cxA = ctx_ps.tile([DV1, 512], F32, tag="cxA")
            cxB = ctx_ps.tile([DV1, 512], F32, tag="cxB")
            for t2 in range(NT2):
                t2s = slice(t2 * P, (t2 + 1) * P)
                sAB = sc_ps.tile([P, 2, 512], F32, tag="s")
                nc.tensor.matmul(
                    sAB[:, 0, :], kt_sb[0:64, pair, t2s], qt_sb[0:64, pair, t1s],
                    start=True, stop=True, tile_position=(0, 0),
                )
                nc.tensor.matmul(
                    sAB[:, 1, :], kt_sb[64:128, pair, t2s], qt_sb[64:128, pair, t1s],
                    start=True, stop=True, tile_position=(64, 0),
                    skip_group_check=True,
                )
                eAB = exp_pool.tile([P, 2, 512], BF16, tag="e")
                nc.scalar.activation(eAB[:, :, :], sAB[:, :, :], AF.Exp, scale=0.125)
                first, last = t2 == 0, t2 == NT2 - 1
                nc.tensor.matmul(
                    cxA[:, :], ve_sb[:, t2, hA, :], eAB[:, 0, :],
                    start=first, stop=last,
                )
                nc.tensor.matmul(
                    cxB[:, :], ve_sb[:, t2, hB, :], eAB[:, 1, :],
                    start=first, stop=last,
                )
                drain_slot()
            # evict unnormalized ctx (bf16) and gather Z rows (f32)
            for h, cx in ((hA, cxA), (hB, cxB)):
                cu = cxu_pool.tile([64, 512], BF16, tag="cu")
                nc.vector.tensor_copy(cu[:], cx[0:64, :])
                zst = work.tile([P, 512], F32, tag="zst")
                nc.vector.tensor_copy(zst[64:65, :], cx[64:65, :])
                nc.gpsimd.dma_start(zall[h:h + 1, :], zst[64:65, :])
                cxu[h] = cu
            if t1b == 0:
                if pair <= 1:
                    # pair+2's K/Q drain during pair+1 (one pair of lead)
                    nxt = pair + 2
                    for nb in range(NBS):
                        fills.append(lambda t=nxt, nb=nb: proj_k(t, nb))
                    fills.append(lambda t=nxt: proj_q(t, 0))
                elif pair == 2:
                    # xT's last readers (K(3,*)) drain during pair 2; then W1
                    # (shares xT slot), Q(*,1), W2 (shares xTo slot)
                    fills.append(load_w1)
                    for t in range(KD):
                        fills.append(lambda t=t: proj_q(t, 1))
                    fills.append(load_w2)
        # rz = 64/Z for all 8 heads at once (ACT, same table set as Exp)
        lz = norm.tile([8, 512], F32, tag="lz")
        nc.scalar.activation(lz[:], zall[:], AF.Ln)
        rzall = norm.tile([8, 512], BF16, tag=f"rzall{t1b % 2}")
        nc.scalar.activation(
            rzall[:], lz[:], AF.Exp, scale=-1.0, bias=ln64_sb[:, 0:1]
        )
        if t1b == 0:
            # normalize runs at the front of the block-1 shadow (the queue is
            # empty here; block-1 fills are appended after these)
            assert not fills
            for h in range(H):
                fills.append(lambda h=h, t1s=t1s, rz=rzall, cx=dict(cxu):
                             z_normalize(h, t1s, rz, cx))
        else:
            for h in range(H):
                z_normalize(h, t1s, rzall, cxu, in_tail=True)
                if fills:
                    drain(2)
                else:
                    wps = scratch_ps([P, 512], F32)
                    nc.tensor.matmul(wps[:], scrap[:, 0:128], scrap[:],
                                     start=True, stop=True)

    # tail: block-1 post + FFN (block-0's ffn2 t=2,3 finishers overlap)
    drain(len(fills))
    ffn2_fin(2, eng=nc.gpsimd)
    post_attn1(4)
    post_attn1(5)
    ffn2_fin(3, eng=nc.gpsimd)
    post_attn1(6)
    post_attn1(7)
    for t in range(4, 8):
        post_attn2a(t, eng=nc.vector)
        # keep-warm matmuls: the LN chain leaves the PE idle here and a
        # ~3.4us idle window re-throttles the clock for the whole FFN tail
        for _ in range(2):
            wps = scratch_ps([P, 512], F32)
            nc.tensor.matmul(wps[:], scrap[:, 0:128], scrap[:],
                             start=True, stop=True)
        for j in range(KD):
            post_attn2b(t, j, evict_act=(j % 2 == 0))
    for m in range(NDFF):
        ffn1(1, m, on_act=(m % 2 == 0))
    for t in range(4, 8):
        ffn2_mm(t, 0, NDFF, acc=("ff" if t % 2 == 0 else "mm"))
        ffn2_fin(t, eng=nc.gpsimd if t < 7 else nc.vector)


def _patch_act_tables():
    """Force every ACT op onto the natural_log_exp_and_others table set so
    the kernel pays one ACT_TABLE_LOAD instead of thrashing between the
    per-function default sets (Exp<->Ln cost 33 loads / 42us)."""
    import functools
    import concourse.hw_specs as hw_specs

    if getattr(hw_specs, "_nle_only", False):
        return
    orig = hw_specs.get_activation_tables

    @functools.cache
    def nle_only(arch):
        tabs = orig(arch)
        return {
            k: (v if k == "natural_log_exp_and_others" else set())
            for k, v in tabs.items()
        }

    hw_specs.get_activation_tables = nle_only
    hw_specs._nle_only = True
    # bacc imported the symbol directly
    if getattr(bacc, "get_activation_tables", None) is not None:
        bacc.get_activation_tables = nle_only


def build_program():
    _patch_act_tables()
    nc = bacc.Bacc("TRN2", target_bir_lowering=False, debug=False, num_devices=NCORES)
    io = {}
    io["xT"] = nc.dram_tensor("xT", [D, S], FP8, kind="ExternalInput").ap()
    io["xTo"] = nc.dram_tensor("xTo", [D, T1], FP8, kind="ExternalInput").ap()
    io["xo"] = nc.dram_tensor("xo", [T1, D], F32, kind="ExternalInput").ap()
    for name, shape in [
        ("wq", [D, D]), ("wk", [D, D]), ("wv", [D, D]), ("wo", [D, D]),
        ("w1", [D, DFF]),
    ]:
        io[name] = nc.dram_tensor(name, shape, FP8, kind="ExternalInput").ap()
    io["w2"] = nc.dram_tensor("w2", [DFF, D], BF16, kind="ExternalInput").ap()
    for name, n in [
        ("bq", D), ("bk", D), ("bv", D), ("b1", DFF), ("b2", D),
        ("g1", D), ("be1", D), ("g2", D), ("be2", D),
    ]:
        io[name] = nc.dram_tensor(name, [n], F32, kind="ExternalInput").ap()
    io["ind8"] = nc.dram_tensor("ind8", [8, D], BF16, kind="ExternalInput").ap()
    io["out"] = nc.dram_tensor("out", [T1, D], F32, kind="ExternalOutput").ap()

    with tile.TileContext(nc) as tc:
        with ExitStack() as ctx:
            emit(ctx, tc, io)
    nc.compile()
    return nc


# feature permutation for the strip-mapped K^T/Q^T layouts: psum tile
# t=(hs,plane), partition q -> head 4*hs + q//32, dk 32*plane + q%32
def _kq_perm():
    perm = np.empty(D, np.int64)
    for t in range(KD):
        hs, plane = divmod(t, 2)
        for q in range(P):
            perm[t * P + q] = (4 * hs + q // 32) * DK + 32 * plane + q % 32
    return perm


def make_in_maps(x, Wq, bq, Wk, bk, Wv, bv, Wo, bo, W1, b1, W2, b2,
                 g1, be1, g2, be2):
    bf = ml_dtypes.bfloat16
    f8 = ml_dtypes.float8_e4m3fn
    f32 = np.float32
    shared = {
        "wq": (np.asarray(Wq, f32) * 0.5).astype(f8),
        "wk": (np.asarray(Wk, f32) * 0.5).astype(f8),
        "wv": (np.asarray(Wv, f32) * 0.5).astype(f8),
        "wo": (np.asarray(Wo, f32) * 4.0).astype(f8),
        "w1": (np.asarray(W1, f32) * 0.5).astype(f8),
        "w2": np.asarray(W2, f32).astype(bf),
        "bq": np.asarray(bq, f32),
        "bk": np.asarray(bk, f32),
        "bv": np.asarray(bv, f32),
        "b1": np.asarray(b1, f32),
        "b2": np.asarray(b2, f32), "g1": np.asarray(g1, f32),
        "be1": np.asarray(be1, f32), "g2": np.asarray(g2, f32),
        "be2": np.asarray(be2, f32),
        "ind8": np.kron(np.eye(H, dtype=f32), np.ones((1, DK), f32)).astype(bf),
    }
    x = np.asarray(x, f32)
    bo_f = np.asarray(bo, f32)
    in_maps = []
    for c in range(NCORES):
        b, half = divmod(c, 2)
        xb = x[b]                                  # [S, D] f32
        xTb = np.ascontiguousarray(xb.T * 2.0).astype(f8)  # [D, S] fp8 = 2x
        sl = slice(half * T1, (half + 1) * T1)
        m = dict(shared)
        m["xT"] = xTb
        m["xTo"] = np.ascontiguousarray(xTb[:, sl])
        m["xo"] = np.ascontiguousarray(xb[sl]) + bo_f
        in_maps.append(m)
    return in_maps


_prog_cache = {}


def get_program():
    if "nc" not in _prog_cache:
        _prog_cache["nc"] = build_program()
    return _prog_cache["nc"]


def kernel(**inputs) -> np.ndarray:
    nc = get_program()
    in_maps = make_in_maps(**inputs)
    res = run_bass_kernel_spmd(nc, in_maps, core_ids=list(range(NCORES)))
    out = np.empty((B, S, D), np.float32)
    for c in range(NCORES):
        b, half = divmod(c, 2)
        out[b, half * T1:(half + 1) * T1] = res.results[c]["out"]
    return out


if __name__ == "__main__":
    rng = np.random.default_rng(0)
    print("building program...")
    get_program()
    print("built")
